# revision 1
# baseline (speedup 1.0000x reference)
"""Bi-directional correlation cost volume on 8 Trainium2 NeuronCores.

Strategy (data-parallel over batch, one batch element per core):
  - Per core, compute the Gram band G[u, x] = sum_c L[c,h,u] * R[c,h,x] / C
    for |x - u| <= 63 with TensorE matmuls (K=C=32, 4x row-tiled over
    h-groups so 4 matmuls share the PE array).
  - Stage the band rectangles to HBM as [h, chunk, u, x-window].
  - The cost volume out[d, x] = G[x -/+ d, x] is a *shear* of the band;
    host extracts the 127 diagonals with one vectorized gather per batch.
"""

import numpy as np

B, C, H, WIMG, D = 8, 32, 160, 320, 64
# (u0, U, xw0, W): u-chunk start/size, x-window start/size
CHUNKS = [(0, 128, 0, 191), (128, 128, 65, 254), (256, 64, 193, 127)]
WSLOT = 256
# staging: per h, chunk ci is a [128, WSLOT] slot; row u holds W valid elems
WPAD = [WSLOT, WSLOT, WSLOT]
COFF = [0, 128 * WSLOT, 2 * 128 * WSLOT]
HROW = 3 * 128 * WSLOT
HQ = H // 4  # h-rows per PE quadrant

_CACHE = {}


HGRP = 16      # h-rows batched per store DMA
ACT_MOD = 3    # every ACT_MOD-th copy goes to ScalarE (0 = all DVE)
STAG_BF16 = False  # stage the Gram band in bf16 (halves store traffic)


def _get_nc(reps=1):
    key = ("nc", reps, HGRP, ACT_MOD, STAG_BF16)
    if key in _CACHE:
        return _CACHE[key]
    import concourse.bacc as bacc
    import concourse.tile as tile
    from concourse import mybir

    f32 = mybir.dt.float32
    sdt = mybir.dt.bfloat16 if STAG_BF16 else f32
    nc = bacc.Bacc("TRN2", target_bir_lowering=False, debug=False)
    r_in = nc.declare_dram_parameter("r_in", [C, H, WIMG], f32, isOutput=False)
    l_in = nc.declare_dram_parameter("l_in", [C, H, WIMG], f32, isOutput=False)
    stag = nc.declare_dram_parameter("stag", [H, HROW], sdt, isOutput=True)

    with tile.TileContext(nc) as tc:
        with tc.tile_pool(name="inp", bufs=1) as inp_pool, \
             tc.tile_pool(name="ps", bufs=6, space="PSUM") as ps_pool, \
             tc.tile_pool(name="st", bufs=6) as st_pool:
            Lsb = inp_pool.tile([128, HQ * WIMG], f32, tag="L")
            Rsb = inp_pool.tile([128, HQ * WIMG], f32, tag="R")
            # partition (q, c) holds h-rows [40q, 40q+40) of channel c
            for q in range(4):
                nc.sync.dma_start(
                    Lsb[32 * q:32 * (q + 1), :],
                    l_in[:, HQ * q:HQ * (q + 1), :].rearrange(
                        "c hh x -> c (hh x)"),
                )
                nc.sync.dma_start(
                    Rsb[32 * q:32 * (q + 1), :],
                    r_in[:, HQ * q:HQ * (q + 1), :].rearrange(
                        "c hh x -> c (hh x)"),
                )
            for _ in range(reps):
                for q in range(4):
                    for hh0 in range(0, HQ, HGRP):
                        G = min(HGRP, HQ - hh0)
                        for ci, (u0, U, xw0, W) in enumerate(CHUNKS):
                            sb = st_pool.tile([128, HGRP * WSLOT], sdt,
                                              tag="sb")
                            for g in range(G):
                                hh = hh0 + g
                                ps = ps_pool.tile([128, 256], f32, tag="ps")
                                nc.tensor.matmul(
                                    ps[:U, :W],
                                    Lsb[32 * q:32 * (q + 1),
                                        hh * WIMG + u0:hh * WIMG + u0 + U],
                                    Rsb[32 * q:32 * (q + 1),
                                        hh * WIMG + xw0:hh * WIMG + xw0 + W],
                                    start=True, stop=True,
                                    tile_position=(32 * q, 0),
                                )
                                dst = sb[:U, g * WSLOT:g * WSLOT + W]
                                if ACT_MOD and hh % ACT_MOD == ACT_MOD - 1:
                                    nc.scalar.mul(dst, ps[:U, :W], 1.0 / C)
                                else:
                                    nc.vector.tensor_scalar_mul(
                                        dst, ps[:U, :W], 1.0 / C)
                            h0 = HQ * q + hh0
                            dma_eng = nc.sync if ci % 2 else nc.scalar
                            dst_ap = stag[h0:h0 + G,
                                          COFF[ci]:COFF[ci] + U * WPAD[ci]]
                            dma_eng.dma_start(
                                dst_ap.rearrange(
                                    "g (u w) -> u g w", u=U)[:, :, :W],
                                sb[:U, :].rearrange(
                                    "u (g w) -> u g w", g=HGRP)[:, :G, :W],
                            )
    nc.compile()
    _CACHE[key] = nc
    return nc


def _gather_idx():
    if "idx" in _CACHE:
        return _CACHE["idx"]
    P_ = np.arange(2 * D)[:, None]
    dts = np.where(P_ < D, P_, -(P_ - D))  # signed disparity per output plane
    x = np.arange(WIMG)[None, :]
    u = np.clip(x - dts, 0, WIMG - 1)
    c = np.minimum(u // 128, 2)
    u0 = c * 128
    xw0 = np.choose(c, [ch[2] for ch in CHUNKS])
    Wc = np.choose(c, [ch[3] for ch in CHUNKS])
    wp = np.choose(c, WPAD)
    off = np.choose(c, COFF)
    w = np.clip(x - xw0, 0, Wc - 1)
    idx2d = off + (u - u0) * wp + w
    _CACHE["idx"] = np.ascontiguousarray(idx2d.reshape(-1).astype(np.int64))
    return _CACHE["idx"]


def _assemble(stag_b):
    """stag_b: [H, HROW] packed band -> out_b [2D, H, WIMG]"""
    idx = _gather_idx()
    flat = np.asarray(stag_b).astype(np.float32).reshape(H, -1)
    o = np.empty((H, 2 * D, WIMG), dtype=np.float32)
    ov = o.reshape(H, -1)
    for h in range(H):
        np.take(flat[h], idx, out=ov[h])
    o = np.ascontiguousarray(o.transpose(1, 0, 2))
    for d in range(1, D):
        o[d, :, :d] = 0
        o[D + d, :, WIMG - d:] = 0
    return o


def run_cores(right_np, left_np, timing_reps=0):
    """Run the SPMD bass kernel; returns (list of staging arrays, exec_ns)."""
    from concourse.bass_utils import run_bass_kernel_spmd

    nc = _get_nc()
    in_maps = [
        {"r_in": np.ascontiguousarray(right_np[b]),
         "l_in": np.ascontiguousarray(left_np[b])}
        for b in range(B)
    ]
    res = run_bass_kernel_spmd(nc, in_maps, list(range(B)))
    return [res.results[b]["stag"] for b in range(B)]


def kernel(right_feature, left_feature, max_disp):
    assert int(max_disp) == D
    right_np = np.asarray(right_feature, dtype=np.float32)
    left_np = np.asarray(left_feature, dtype=np.float32)
    stags = run_cores(right_np, left_np)
    out = np.stack([_assemble(s) for s in stags])
    return out



# revision 2
# speedup vs baseline: 13.0485x; 13.0485x over previous
"""Bi-directional correlation cost volume on 8 Trainium2 NeuronCores.

Strategy (data-parallel over batch, one batch element per core):
  - Host casts both feature maps to bf16 (rel err ~3e-3, gate is 2e-2).
    The PE then computes the Gram band G[u, x] = sum_c L[c,h,u] * R[c,h,x]
    at bf16 rate (1 cycle/row @2.4GHz vs fp32's 4 cycles/row).
  - PSUM (f32) is drained to SBUF as bf16 with the 1/C scale fused in,
    split across DVE and ACT by greedy load balance.
  - The staged band is packed [u, g, w] per (h-group, chunk) so every DMA
    descriptor moves one >=4KB contiguous run per partition row: runs
    under 512B pay a 2x DMA-latency penalty, so layout - not dtype - is
    what unlocks the store bandwidth (20.8MB @ ~360GB/s).
  - The cost volume out[d, x] = G[x -/+ d, x] is a *shear* of the band;
    host extracts the 127 diagonals with one vectorized gather per batch.
"""

import numpy as np

B, C, H, WIMG, D = 8, 32, 160, 320, 64
# (u0, U, xw0, W): u-chunk start/size, x-window start/size
CHUNKS = [(0, 128, 0, 191), (128, 128, 65, 254), (256, 64, 193, 127)]
HQ = H // 4      # 40 h-rows per PE quadrant
HGRP = 20        # h-rows batched per staging tile / store DMA
NHG = H // HGRP  # 8 h-groups
# per-group staging: chunk ci at GOFF[ci], laid out [u, g, w] contiguous
GOFF = [0, 128 * HGRP * 191, 128 * HGRP * (191 + 254)]
GTOT = GOFF[2] + 64 * HGRP * 127

_CACHE = {}


def _get_nc(reps=1):
    key = ("nc", reps)
    if key in _CACHE:
        return _CACHE[key]
    import concourse.bacc as bacc
    import concourse.tile as tile
    from concourse import mybir

    f32 = mybir.dt.float32
    bf16 = mybir.dt.bfloat16
    nc = bacc.Bacc("TRN2", target_bir_lowering=False, debug=False)
    r_in = nc.declare_dram_parameter("r_in", [C, H, WIMG], bf16, isOutput=False)
    l_in = nc.declare_dram_parameter("l_in", [C, H, WIMG], bf16, isOutput=False)
    stag = nc.declare_dram_parameter("stag", [NHG, GTOT], bf16, isOutput=True)

    with tile.TileContext(nc) as tc:
        with tc.tile_pool(name="inp", bufs=1) as inp_pool, \
             tc.tile_pool(name="ps", bufs=8, space="PSUM") as ps_pool, \
             tc.tile_pool(name="st", bufs=6) as st_pool:
            Lsb = inp_pool.tile([128, HQ * WIMG], bf16, tag="L")
            Rsb = inp_pool.tile([128, HQ * WIMG], bf16, tag="R")
            # partition (q, c) holds h-rows [40q, 40q+40) of channel c
            for q in range(4):
                nc.sync.dma_start(
                    Lsb[32 * q:32 * (q + 1), :],
                    l_in[:, HQ * q:HQ * (q + 1), :].rearrange(
                        "c hh x -> c (hh x)"),
                )
                nc.sync.dma_start(
                    Rsb[32 * q:32 * (q + 1), :],
                    r_in[:, HQ * q:HQ * (q + 1), :].rearrange(
                        "c hh x -> c (hh x)"),
                )
            # greedy copy-engine balance: projected busy ns per engine
            eng_ns = {"v": 0.0, "s": 0.0}
            cyc = {"v": 1.0 / 0.96, "s": 1.0 / 1.2}
            for _ in range(reps):
                for q in range(4):
                    for hh0 in range(0, HQ, HGRP):
                        hg = 2 * q + hh0 // HGRP
                        for ci, (u0, U, xw0, W) in enumerate(CHUNKS):
                            sb = st_pool.tile([128, HGRP * W], bf16, tag="sb")
                            for g in range(HGRP):
                                hh = hh0 + g
                                ps = ps_pool.tile([128, 256], f32, tag="ps")
                                nc.tensor.matmul(
                                    ps[:U, :W],
                                    Lsb[32 * q:32 * (q + 1),
                                        hh * WIMG + u0:hh * WIMG + u0 + U],
                                    Rsb[32 * q:32 * (q + 1),
                                        hh * WIMG + xw0:hh * WIMG + xw0 + W],
                                    start=True, stop=True,
                                    tile_position=(32 * q, 0),
                                )
                                dst = sb[:U, g * W:(g + 1) * W]
                                e = "v" if (eng_ns["v"] + W * cyc["v"]
                                            <= eng_ns["s"] + W * cyc["s"]) \
                                    else "s"
                                eng_ns[e] += W * cyc[e]
                                if e == "s":
                                    nc.scalar.mul(dst, ps[:U, :W], 1.0 / C)
                                else:
                                    nc.vector.tensor_scalar_mul(
                                        dst, ps[:U, :W], 1.0 / C)
                            dst_ap = stag[hg, GOFF[ci]:
                                          GOFF[ci] + U * HGRP * W]
                            nc.sync.dma_start(
                                dst_ap.rearrange("(u k) -> u k", u=U),
                                sb[:U, :],
                            )
    nc.compile()
    _CACHE[key] = nc
    return nc


def _gather_idx():
    """GIDX[p, h, x]: flat index into stag.ravel() for output plane p."""
    if "idx" in _CACHE:
        return _CACHE["idx"]
    P_ = np.arange(2 * D)[:, None, None]
    dts = np.where(P_ < D, P_, -(P_ - D))  # signed disparity per plane
    X = np.arange(WIMG)[None, None, :]
    u = np.clip(X - dts, 0, WIMG - 1)      # [2D, 1, W]
    ci = np.minimum(u // 128, 2)
    u0 = ci * 128
    xw0 = np.choose(ci, [c[2] for c in CHUNKS])
    Wc = np.choose(ci, [c[3] for c in CHUNKS])
    off = np.choose(ci, GOFF)
    w = X - xw0
    base = off + (u - u0) * (HGRP * Wc) + w  # [2D, 1, W]
    Hh = np.arange(H)[None, :, None]
    qq, rem = Hh // HQ, Hh % HQ
    hg = 2 * qq + rem // HGRP
    g = rem % HGRP
    gidx = hg * GTOT + base + g * Wc         # [2D, H, W]
    _CACHE["idx"] = np.ascontiguousarray(gidx.astype(np.int64))
    return _CACHE["idx"]


def _assemble(stag_b):
    """stag_b: [NHG, GTOT] packed bf16 band -> out_b [2D, H, WIMG] f32"""
    idx = _gather_idx()
    flat = np.asarray(stag_b).astype(np.float32).ravel()
    o = np.take(flat, idx)
    for d in range(1, D):
        o[d, :, :d] = 0
        o[D + d, :, WIMG - d:] = 0
    return o


def run_cores(right_np, left_np, timing_reps=0):
    """Run the SPMD bass kernel; returns list of per-core staging arrays."""
    import ml_dtypes
    from concourse.bass_utils import run_bass_kernel_spmd

    nc = _get_nc()
    bf = ml_dtypes.bfloat16
    in_maps = [
        {"r_in": np.ascontiguousarray(right_np[b].astype(bf)),
         "l_in": np.ascontiguousarray(left_np[b].astype(bf))}
        for b in range(B)
    ]
    res = run_bass_kernel_spmd(nc, in_maps, list(range(B)))
    return [res.results[b]["stag"] for b in range(B)]


def kernel(right_feature, left_feature, max_disp):
    assert int(max_disp) == D
    right_np = np.asarray(right_feature, dtype=np.float32)
    left_np = np.asarray(left_feature, dtype=np.float32)
    stags = run_cores(right_np, left_np)
    out = np.stack([_assemble(s) for s in stags])
    return out
